# revision 5
# baseline (speedup 1.0000x reference)
"""Multi-layer tanh RNN on 8 Trainium2 NeuronCores.

Strategy (data-parallel over batch, 16 rows/core):
- Wavefront over the 4 layers: at wavefront s, layer j processes t = s - j.
  The 4 (layer, t) units are independent and run CONCURRENTLY in the PE
  array via 4-way column tiling (tile_position=(0, 32j)).
- Matmuls are bf16 with hi/lo compensation ("x2" scheme): each col group's
  stationary operand is [h_hi | h_lo] (2x16 = 32 columns), both multiplying
  the same bf16 weight stream. The hi and lo partial outputs land in
  separate PSUM rows and are summed during post-processing, recovering
  ~fp16-level activation precision at zero extra stream cost.
- Weights (W_ih^T and W_hh^T concatenated per layer, K = 2048) are bf16,
  fully SBUF-resident (16 MB).
- Matmul output is batch-major [batch, H]; it is transposed back to H-major
  [H, batch] with PE transpose-mode so that tanh + per-partition bias and
  the next step's stationary loads are in the right layout.
- Sequence loop is a hardware For_i over wavefronts (parity-unrolled by 2).

Host side: shards batch, pre-transposes x to [T, H, B_c] hi/lo bf16, and
re-assembles the output.
"""
import numpy as np
import ml_dtypes

import concourse.bass as bass
import concourse.bacc as bacc
import concourse.mybir as mybir
from concourse import tile
from concourse.bass_utils import run_bass_kernel_spmd

F32 = mybir.dt.float32
BF16 = mybir.dt.bfloat16

SEQ, BATCH, HID, LAYERS = 512, 128, 1024, 4
NCORES = 8
BC = BATCH // NCORES          # 16 batch rows per core
CH = HID // 128               # 8 H-chunks
KT = 2 * HID // 128           # 16 K-tiles (x-part 0..7, h-part 8..15)
XPAD = 4                      # zero-padded extra timesteps for x prefetch


def build_kernel(repeat: int = 1):
    nc = bacc.Bacc("TRN2", target_bir_lowering=False, debug=False)

    d_xhi = nc.dram_tensor("x_hi", (SEQ + XPAD, HID, BC), BF16,
                           kind="ExternalInput").ap()
    d_xlo = nc.dram_tensor("x_lo", (SEQ + XPAD, HID, BC), BF16,
                           kind="ExternalInput").ap()
    d_w = nc.dram_tensor("w_hi", (LAYERS, 2 * HID, HID), BF16,
                         kind="ExternalInput").ap()
    d_bias = nc.dram_tensor("bias_pc", (128, LAYERS, CH), F32,
                            kind="ExternalInput").ap()
    d_ehi = nc.dram_tensor("e_hi", (128, 64), F32,
                           kind="ExternalInput").ap()
    d_elo = nc.dram_tensor("e_lo", (128, 64), F32,
                           kind="ExternalInput").ap()
    d_out = nc.dram_tensor("outT", (SEQ, HID, BC), F32,
                           kind="ExternalOutput").ap()

    # DRAM views tiled for DMA: [T, H, B] -> [T, chunk, part, B]
    v_xhi = d_xhi.rearrange("t (c p) b -> t c p b", p=128)
    v_xlo = d_xlo.rearrange("t (c p) b -> t c p b", p=128)
    v_w = d_w.rearrange("l (k p) n -> l k p n", p=128)
    v_out = d_out.rearrange("t (c p) b -> t c p b", p=128)

    with tile.TileContext(nc) as tc:
        with (
            tc.tile_pool(name="sbw", bufs=1) as pw,
            tc.tile_pool(name="sbs", bufs=1) as ps,
            tc.tile_pool(name="psA", bufs=1, space="PSUM") as ppa,
            tc.tile_pool(name="psB", bufs=1, space="PSUM") as ppb,
        ):
            # weights: [128, layer, ktile, H]  (128 KB/partition)
            w_sb = pw.tile([128, LAYERS, KT, HID], BF16)
            # h stationaries: [128, parity, layer, chunk, hi16|lo16]
            hbuf = ps.tile([128, 2, LAYERS, CH, 2 * BC], BF16)
            # x stationaries: [128, parity, chunk, hi16|lo16]
            xbuf = ps.tile([128, 2, CH, 2 * BC], BF16)
            # psum evacuation staging (batch-major pre-activations)
            stg = ps.tile([128, 2, HID], F32)
            # tanh output fp32, H-major: [128, parity, chunk, 4*16]
            htmp = ps.tile([128, 2, CH, 4 * BC], F32)
            bias_sb = ps.tile([128, LAYERS, CH], F32)
            ehi_sb = ps.tile([128, 64], F32)
            elo_sb = ps.tile([128, 64], F32)

            psum_mm = [ppa.tile([128, HID], F32, tag=f"pmm{i}", name=f"pmm{i}")
                       for i in range(2)]
            psumT = [ppb.tile([128, CH * 4 * BC], F32, tag=f"pT{i}",
                              name=f"pT{i}") for i in range(2)]

            # ---- init ----
            for l in range(LAYERS):
                nc.sync.dma_start(out=w_sb[:, l], in_=v_w[l].transpose([1, 0, 2]))
            nc.sync.dma_start(out=bias_sb[:], in_=d_bias)
            nc.sync.dma_start(out=ehi_sb[:], in_=d_ehi)
            nc.sync.dma_start(out=elo_sb[:], in_=d_elo)
            nc.vector.memset(hbuf[:], 0.0)
            nc.vector.memset(htmp[:], 0.0)
            nc.vector.memset(stg[:], 0.0)

            def tslice(v, t):
                a = v[t]
                if a.ndim == 4:
                    a = a.squeeze(0)
                return a.transpose([1, 0, 2])

            def dma_x(t_idx, parity):
                """Load x[t] (hi and lo) into xbuf parity slot."""
                nc.sync.dma_start(out=xbuf[:, parity, :, 0:BC],
                                  in_=tslice(v_xhi, t_idx))
                nc.sync.dma_start(out=xbuf[:, parity, :, BC:2 * BC],
                                  in_=tslice(v_xlo, t_idx))

            def stationary(g, k, p):
                """lhsT [128, 32] for unit g, K-tile k, current parity p."""
                if k < CH:  # input part: x for layer 0, h_{g-1} otherwise
                    if g == 0:
                        return xbuf[:, p, k, :]
                    return hbuf[:, 1 - p, g - 1, k, :]
                return hbuf[:, 1 - p, g, k - CH, :]

            def wavefront(p, units, out_t=None, x_t=None, prefetch_t=None,
                          out_units=None):
                """Emit one wavefront.

                p: parity (0/1). units: active unit (=layer) list.
                out_t: DRAM index expr for the unit-3 output DMA (or None).
                x_t: synchronous x load for this wavefront (prologue only).
                prefetch_t: x load for wavefront +2 (steady state).
                out_units: units whose postproc should write hbuf (defaults
                  to `units`).
                """
                if out_units is None:
                    out_units = units
                if x_t is not None:
                    dma_x(x_t, p)
                pm = psum_mm[p]
                pt = psumT[p]
                full = len(units) == 4
                # matmul streams: halves outer, K-tiles, groups inner
                for h in range(2):
                    for k in range(KT):
                        for g in units:
                            nc.tensor.matmul(
                                pm[32 * g:32 * g + 2 * BC,
                                   512 * h:512 * (h + 1)],
                                stationary(g, k, p),
                                w_sb[:, g, k, 512 * h:512 * (h + 1)],
                                start=(k == 0), stop=(k == KT - 1),
                                tile_position=(0, 32 * g),
                            )
                    # postproc for the 4 chunks of this half
                    for c in range(4 * h, 4 * h + 4):
                        lo_c, hi_c = 128 * c, 128 * (c + 1)
                        if full:
                            nc.vector.tensor_copy(stg[:, p, lo_c:hi_c],
                                                  pm[:, lo_c:hi_c])
                        else:
                            for g in units:
                                nc.vector.tensor_copy(
                                    stg[32 * g:32 * g + 2 * BC, p, lo_c:hi_c],
                                    pm[32 * g:32 * g + 2 * BC, lo_c:hi_c])
                        # transpose hi and lo row-sets into the SAME
                        # psum region; the second matmul accumulates -> the
                        # hi+lo combine happens in PSUM for free.
                        nc.tensor.matmul(pt[:, 64 * c:64 * (c + 1)],
                                         stg[:, p, lo_c:hi_c], ehi_sb[:],
                                         is_transpose=True,
                                         start=True, stop=False)
                        nc.tensor.matmul(pt[:, 64 * c:64 * (c + 1)],
                                         stg[:, p, lo_c:hi_c], elo_sb[:],
                                         is_transpose=True,
                                         start=False, stop=True)
                        for g in units:
                            nc.scalar.activation(
                                htmp[:, p, c, BC * g:BC * (g + 1)],
                                pt[:, 64 * c + BC * g:64 * c + BC * (g + 1)],
                                mybir.ActivationFunctionType.Tanh,
                                bias=bias_sb[:, g, c:c + 1])
                        # cast to bf16 hi + lo residual into hbuf
                        for g in out_units:
                            nc.vector.tensor_copy(
                                hbuf[:, p, g, c, 0:BC],
                                htmp[:, p, c, BC * g:BC * (g + 1)])
                            nc.vector.tensor_sub(
                                hbuf[:, p, g, c, BC:2 * BC],
                                htmp[:, p, c, BC * g:BC * (g + 1)],
                                hbuf[:, p, g, c, 0:BC])
                if out_t is not None:
                    nc.sync.dma_start(out=tslice(v_out, out_t),
                                      in_=htmp[:, p, :, 3 * BC:4 * BC])
                if prefetch_t is not None:
                    dma_x(prefetch_t, p)

            import contextlib

            rep_ctx = (tc.For_i(0, repeat, 1) if repeat > 1
                       else contextlib.nullcontext())
            with rep_ctx:
                if repeat > 1:
                    nc.vector.memset(hbuf[:], 0.0)
                # prologue s = 0..3
                wavefront(0, [0], x_t=0)
                wavefront(1, [0, 1], x_t=1)
                wavefront(0, [0, 1, 2], x_t=2)
                wavefront(1, [0, 1, 2, 3], x_t=3, out_t=0)
                dma_x(4, 0)
                dma_x(5, 1)
                # steady state s = 4..511 (parity-unrolled by 2)
                with tc.For_i(4, SEQ, 2, hint_engines=(mybir.EngineType.PE,),
                              name="wf") as s:
                    wavefront(0, [0, 1, 2, 3], out_t=bass.ds(s - 3, 1),
                              prefetch_t=bass.ds(s + 2, 1))
                    wavefront(1, [0, 1, 2, 3], out_t=bass.ds(s - 2, 1),
                              prefetch_t=bass.ds(s + 3, 1))
                # epilogue s = 512..514
                wavefront(0, [1, 2, 3], out_t=SEQ - 3)
                wavefront(1, [2, 3], out_t=SEQ - 2)
                wavefront(0, [3], out_t=SEQ - 1)

    nc.compile()
    return nc


def _prep_inputs(x, W_ih, W_hh, b_ih, b_hh):
    """Host-side prep shared across cores + per-core shards."""
    bf16 = ml_dtypes.bfloat16
    # weights: concat [W_ih^T; W_hh^T] per layer -> [L, 2H, H] bf16
    w = np.empty((LAYERS, 2 * HID, HID), dtype=bf16)
    for l in range(LAYERS):
        w[l, :HID] = W_ih[l].T.astype(bf16)
        w[l, HID:] = W_hh[l].T.astype(bf16)
    bias = (b_ih.astype(np.float64) + b_hh.astype(np.float64)).astype(np.float32)
    # [L, H] -> [128, L, CH] partition-major
    bias_pc = np.ascontiguousarray(
        bias.reshape(LAYERS, CH, 128).transpose(2, 0, 1))
    e_hi = np.zeros((128, 64), dtype=np.float32)
    e_lo = np.zeros((128, 64), dtype=np.float32)
    for j in range(64):
        g, b = j // BC, j % BC
        e_hi[32 * g + b, j] = 1.0
        e_lo[32 * g + BC + b, j] = 1.0

    shards = []
    for c in range(NCORES):
        xs = x[:, BC * c:BC * (c + 1), :]           # [T, BC, H]
        xT = np.ascontiguousarray(xs.transpose(0, 2, 1)).astype(np.float32)
        x_hi = np.zeros((SEQ + XPAD, HID, BC), dtype=bf16)
        x_lo = np.zeros((SEQ + XPAD, HID, BC), dtype=bf16)
        hi = xT.astype(bf16)
        x_hi[:SEQ] = hi
        x_lo[:SEQ] = (xT - hi.astype(np.float32)).astype(bf16)
        shards.append({"x_hi": x_hi, "x_lo": x_lo, "w_hi": w,
                       "bias_pc": bias_pc, "e_hi": e_hi, "e_lo": e_lo})
    return shards


def kernel(x, W_ih, W_hh, b_ih, b_hh):
    x = np.asarray(x, dtype=np.float32)
    shards = _prep_inputs(x, np.asarray(W_ih), np.asarray(W_hh),
                          np.asarray(b_ih), np.asarray(b_hh))
    nc = build_kernel(repeat=1)
    res = run_bass_kernel_spmd(nc, shards, core_ids=list(range(NCORES)),
                               trace=False)
    out = np.empty((SEQ, BATCH, HID), dtype=np.float32)
    for c in range(NCORES):
        outT = res.results[c]["outT"]               # [T, H, BC]
        out[:, BC * c:BC * (c + 1), :] = outT.transpose(0, 2, 1)
    return out
